# revision 28
# baseline (speedup 1.0000x reference)
"""MultiHeadAttention TRN2 kernel (B=2, L=2048, D=1024, H=16).

Sharding: 8 cores = 2 batches x 4 head-groups (4 heads each).
Each core computes Q/K/V projections for its 4 heads, attention, and a
partial out-projection (its heads' slice of Wo). Host sums the 4
partials per batch and adds bo.

v3 design notes
---------------
The kernel is bound by the Activation engine: softmax Exp touches
4 heads x 2048^2 = 16.8M elements/core at 1 elem/lane/cycle => ~130us
of irreducible ACT work. Everything is scheduled around starting the
Exp stream as early as possible (right after the K projection + the
first Q-projection block) and never letting it starve:

- Emission order = per-engine execution order (queues are in-order), so
  projections/PV/out-proj work is interleaved into the attention loop
  at ~2.1us/iter pacing instead of being emitted phase-by-phase.
- PSUM (8 banks): scores 2x[128,2,512] (4) + pv pair [128,2,512] (2)
  + rowsums d [128,512] (1) + filler slot [128,512] (1).
- PV uses M=64 col-tiled matmul pairs (tile_position (0,0)/(0,64)), so
  both heads of a pair run concurrently in the PE array and the
  normalized output lands partition-aligned for the whole pair.
- Softmax denominators accumulate in a separate 1-bank PSUM tile via
  M=1 ones-matmuls col-tiled to partitions {0,32,64,96}.
- Reciprocal rows are broadcast 1->64 partitions with K=1 ones-matmuls
  (col-tiled pair), then one fused multiply+bias per ql.

Per-core layouts (all transposes done host-side, free):
  xq/xk/xv : [8, 128, 2048] xdt  = x[b].T d-blocks           (d, tok)
  wq/wk/wv : [128, 8, 256]  xdt  = W_slice.T blocks          (d, feat)
  wo       : [128, 2, 1024] f32r = Wo[:, fslice].T blocks    (feat, dout)
  bv       : [128, 2]       f32  = bv slice as (pair, 128).T
  out      : [8, 128, 2048] f32  = partial_out.T d-blocks    (dout, tok)
"""

import numpy as np

B, L, D, H = 2, 2048, 1024, 16
HD = 64
NH = H // 4  # heads per core = 4
F = NH * HD  # 256 feats per core
NCORES = 8

XDT = "bf16"  # input dtype for x / qkv-weights: "bf16" or "f32r"
FP8QK = True  # fp8e4m3 + DoubleRow for the Q/K projections (x and W
              # pre-scaled by 32; compensated in the softmax exp scale)

_CACHE = {}


def _build(xdt_name=XDT, fp8qk=FP8QK, debug=False, repeat=1, loop_n=None,
           ablate=None):
    from contextlib import ExitStack

    import concourse.tile as tile
    from concourse import bacc, mybir

    F32 = mybir.dt.float32
    I16 = mybir.dt.int16
    FP8 = mybir.dt.float8e4
    F32R = mybir.dt.float32r
    BF16 = mybir.dt.bfloat16
    XDT_ = mybir.dt.bfloat16 if xdt_name == "bf16" else F32R
    AF = mybir.ActivationFunctionType

    KB = D // 128  # 8 contraction blocks
    NQC = L // 512  # 4 query chunks
    NKT = L // 128  # 16 key tiles
    NMT = L // 128  # 16 token tiles (V rows)
    NFB = F // 128  # 2 feature blocks
    NDB = D // 128  # 8 dout blocks

    nc = bacc.Bacc(
        "TRN2", target_bir_lowering=False, debug=False, enable_asserts=False
    )

    QKD = FP8 if fp8qk else XDT_
    if fp8qk:
        xq = nc.dram_tensor("xq", [KB // 2, 128, 2, L], FP8,
                            kind="ExternalInput").ap()
        xk = nc.dram_tensor("xk", [KB // 2, 128, 2, L], FP8,
                            kind="ExternalInput").ap()
    else:
        xq = nc.dram_tensor("xq", [KB, 128, L], XDT_,
                            kind="ExternalInput").ap()
        xk = nc.dram_tensor("xk", [KB, 128, L], XDT_,
                            kind="ExternalInput").ap()
    xv = nc.dram_tensor("xv", [KB, 128, L], XDT_, kind="ExternalInput").ap()
    wq = nc.dram_tensor("wq", [128, KB, F], QKD, kind="ExternalInput").ap()
    wk = nc.dram_tensor("wk", [128, KB, F], QKD, kind="ExternalInput").ap()
    wv = nc.dram_tensor("wv", [128, KB, F], XDT_, kind="ExternalInput").ap()
    wo = nc.dram_tensor("wo", [128, NFB, D], F32R, kind="ExternalInput").ap()
    bq = nc.dram_tensor("bq", [128, NFB], F32, kind="ExternalInput").ap()
    bk = nc.dram_tensor("bk", [128, NFB], F32, kind="ExternalInput").ap()
    bv = nc.dram_tensor("bv", [64, NH], F32, kind="ExternalInput").ap()
    out = nc.dram_tensor("out", [NDB, 128, L], BF16,
                         kind="ExternalOutput").ap()

    with tile.TileContext(nc) as tc, ExitStack() as ctx:
        wp = ctx.enter_context(tc.tile_pool(name="wp", bufs=1))
        xt = ctx.enter_context(tc.tile_pool(name="xt", bufs=9))
        qk = ctx.enter_context(tc.tile_pool(name="qk", bufs=1))
        vpp = ctx.enter_context(tc.tile_pool(name="vpp", bufs=16))
        otp = ctx.enter_context(tc.tile_pool(name="otp", bufs=2))
        ep = ctx.enter_context(tc.tile_pool(name="ep", bufs=24))
        rp = ctx.enter_context(tc.tile_pool(name="rp", bufs=2))
        osp = ctx.enter_context(tc.tile_pool(name="osp", bufs=3))
        ps = ctx.enter_context(tc.tile_pool(name="ps", bufs=2, space="PSUM"))

        # --- weights ---
        wq_s = wp.tile([128, KB, F], QKD, tag="wq")
        wk_s = wp.tile([128, KB, F], QKD, tag="wk")
        wv_s = wp.tile([128, KB, F], XDT_, tag="wv")
        wo_s = wp.tile([128, NFB, D], F32R, tag="wo")
        bq_s = wp.tile([128, NFB], F32, tag="bq")
        bk_s = wp.tile([128, NFB], F32, tag="bk")
        bv_s = wp.tile([64, NH], F32, tag="bv")
        ones_s = wp.tile([1, 64], F32R, tag="ones")
        for t, d in [(bq_s, bq), (bk_s, bk), (bv_s, bv),
                     (wk_s, wk), (wq_s, wq)]:
            nc.sync.dma_start(t[:], d)
        nc.vector.tensor_scalar(
            ones_s[:], bq_s[0:1, 0:1].broadcast_to([1, 64]),
            0.0, 1.0, mybir.AluOpType.mult, mybir.AluOpType.add,
        )

        import contextlib
        loop_ctx = tc.For_i(0, loop_n, 1) if loop_n else contextlib.nullcontext()
        with loop_ctx:
         for _rep in range(repeat):
            # ---- input DMAs (xk, xq first: scores gate the Exp stream) ----
            xk_t, xq_t, xv_t = [], [], []
            skip_in_dma = (ablate == "nodma")
            nqk = KB // 2 if fp8qk else KB
            for kb in range(nqk):
                if fp8qk:
                    t = xt.tile([128, 2, L], FP8, tag="x8", bufs=8,
                                name=f"xk_{kb}")
                else:
                    t = xt.tile([128, L], XDT_, tag="xt", name=f"xk_{kb}")
                if not skip_in_dma:
                    nc.sync.dma_start(t[:], xk[kb])
                xk_t.append(t)
            for kb in range(nqk):
                if fp8qk:
                    t = xt.tile([128, 2, L], FP8, tag="x8", bufs=8,
                                name=f"xq_{kb}")
                else:
                    t = xt.tile([128, L], XDT_, tag="xt", name=f"xq_{kb}")
                if not skip_in_dma:
                    # third DMA ring (ACT queue, empty at startup): xq
                    # streams in parallel with xk instead of behind it
                    nc.scalar.dma_start(t[:], xq[kb])
                xq_t.append(t)
            # V/out-proj weights are not needed until mid-attention: keep
            # them behind xk/xq so the first scores land ~8us earlier
            nc.gpsimd.dma_start(wv_s[:], wv)
            nc.gpsimd.dma_start(wo_s[:], wo)
            for kb in range(KB):
                t = xt.tile([128, L], XDT_, tag="xt", name=f"xv_{kb}")
                if not skip_in_dma:
                    nc.gpsimd.dma_start(t[:], xv[kb])
                xv_t.append(t)

            if ablate == "dmaonly":
                for i in range(2 * NDB * 2):
                    qp, r = i // (2 * NDB), i % (2 * NDB)
                    ql, mt = r // NDB, r % NDB
                    og = osp.tile([128, 512], BF16, tag="og",
                                  name=f"ogd{i}")
                    nc.vector.memset(og[:], 0)
                    nc.sync.dma_start(
                        out[mt][:, (qp * 2 + ql) * 512:
                                (qp * 2 + ql + 1) * 512], og[:])
                continue

            # ---- K projection (all), Q projection fb=0 ----
            kt_t, qt_t = {}, {}

            def proj_group(which, fb, qc, slot_tag="acc"):
                w_s, b_s, x_t, dst = (
                    (wk_s, bk_s, xk_t, kt_t) if which == "k" else
                    (wq_s, bq_s, xq_t, qt_t)
                )
                pa = ps.tile([128, 512], F32, tag="acc",
                             name=f"pa_{which}{fb}{qc}")
                if fp8qk:
                    for g in range(KB // 2):
                        nc.tensor.matmul(
                            pa[:],
                            w_s[:, 2 * g:2 * g + 2, fb * 128:(fb + 1) * 128],
                            x_t[g][:, :, qc * 512:(qc + 1) * 512],
                            start=(g == 0),
                            stop=(g == KB // 2 - 1),
                            perf_mode=mybir.MatmulPerfMode.DoubleRow,
                        )
                else:
                    for kb in range(KB):
                        nc.tensor.matmul(
                            pa[:],
                            w_s[:, kb, fb * 128:(fb + 1) * 128],
                            x_t[kb][:, qc * 512:(qc + 1) * 512],
                            start=(kb == 0),
                            stop=(kb == KB - 1),
                        )
                t = qk.tile([128, 512], BF16, tag=f"{which}t{fb}{qc}",
                            name=f"{which}t{fb}{qc}")
                nc.vector.tensor_scalar_add(t[:], pa[:], b_s[:, fb:fb + 1])
                dst[(fb, qc)] = t

            # minimal prefix: first scores need only K(0,0) + Q(0,0/1);
            # remaining groups are deadline-scheduled into early iters
            proj_group("k", 0, 0)
            proj_group("q", 0, 0)
            proj_group("q", 0, 1)
            proj_pending = [("k", 0, 1), ("k", 0, 2), ("k", 0, 3),
                            ("k", 1, 0), ("k", 1, 1), ("k", 1, 2),
                            ("k", 1, 3), ("q", 0, 2), ("q", 0, 3),
                            ("q", 1, 0), ("q", 1, 1), ("q", 1, 2),
                            ("q", 1, 3)]

            # ---- V projection group (emitted as filler inside attention) --
            vp_t = [None] * NMT

            def v_group(mt):
                pb = ps.tile([128, 512], F32, tag="acc", name=f"pb{mt}")
                for kb in range(KB):
                    nc.tensor.matmul(
                        pb[:, 0:F],
                        xv_t[kb][:, mt * 128:(mt + 1) * 128],
                        wv_s[:, kb, :],
                        start=(kb == 0),
                        stop=(kb == KB - 1),
                    )
                v = vpp.tile([128, NH, 65], BF16, tag="vp", name=f"vp{mt}")
                # ones column (col 64): x*0 + 1 — memset can't emit f32r
                nc.vector.tensor_scalar(
                    v[:, :, 64], bq_s[:, 0:1].broadcast_to([128, NH]),
                    0.0, 1.0, mybir.AluOpType.mult, mybir.AluOpType.add,
                )
                nc.vector.tensor_copy(
                    v[:, :, 0:64],
                    pb[:, 0:F].rearrange("p (h f) -> p h f", h=NH),
                )
                vp_t[mt] = v

            # ---- attention blocks --------------------------------------
            # state per block index blk = 2*qp + pp
            ot_t = [None, None]          # per qp
            pvp_t = [None] * 4           # per blk
            e_t = {}                     # (blk, kt, hh) -> e tile
            og_t = {}                    # (qp, ql, mt) -> og tile

            def block_open(blk):
                qp, pp = blk // 2, blk % 2
                if pp == 0:
                    ot_t[qp] = otp.tile([128, NFB, 2, 512], F32R, tag="ot",
                                    name=f"ot{qp}")
                pvp_t[blk] = [
                    ps.tile([65, 2, 512], F32, tag="pv",
                            name=f"pv{blk}_{i}")
                    for i in range(2)
                ]

            def scores_exp(blk, kt):
                qp, pp = blk // 2, blk % 2
                s = [ps.tile([128, 2, 512], F32, tag="acc", name=f"s{blk}_{kt}_{i}")
                     for i in range(2)]
                for hh in range(2):
                    pl, ph = 64 * hh, 64 * (hh + 1)
                    for ql in range(2):
                        qc = qp * 2 + ql
                        nc.tensor.matmul(
                            s[hh][:, ql, :],
                            kt_t[(pp, kt // 4)][pl:ph,
                                                (kt % 4) * 128:
                                                (kt % 4 + 1) * 128],
                            qt_t[(pp, qc)][pl:ph, :],
                            start=True,
                            stop=True,
                        )
                for hh in range(2):
                    et = ep.tile([128, 2, 512], BF16, tag="e", name=f"e{blk}_{kt}_{hh}")
                    esc = 0.125 / 1024.0 if fp8qk else 0.125
                    if hh == 1 and (kt % 2) == 0:
                        # bf16 Schraudolph exp on DVE: bitcast-int16 of
                        # round(s*(128/(8*ln2)) + b16) ~= bf16(exp(s/8)),
                        # zero-mean-bias constant; offloads the saturated
                        # ACT engine (max rel err ~4%, softmax-consistent)
                        nc.vector.tensor_scalar(
                            et[:].bitcast(I16), s[hh][:],
                            23.0831450857 * (esc * 8.0), 16248.5,
                            mybir.AluOpType.mult, mybir.AluOpType.add,
                        )
                    else:
                        nc.scalar.activation(et[:], s[hh][:], AF.Exp,
                                             scale=esc)
                    e_t[(blk, kt, hh)] = et

            def pv_d(blk, kt):
                qp, pp = blk // 2, blk % 2
                pv = pvp_t[blk]
                st, sp = (kt == 0), (kt == NKT - 1)
                for hh in range(2):
                    for ql in range(2):
                        nc.tensor.matmul(
                            pv[hh][:, ql, :],
                            vp_t[kt][:, 2 * pp + hh, :],
                            e_t[(blk, kt, hh)][:, ql, :],
                            start=st, stop=sp,
                        )
                if sp:
                    for hh in range(2):
                        for kk in range(NKT):
                            del e_t[(blk, kk, hh)]

            def normalize(blk):
                qp, pp = blk // 2, blk % 2
                pv, ot = pvp_t[blk], ot_t[qp]
                for hh in range(2):
                    h = 2 * pp + hh
                    rs = rp.tile([1, 2, 512], F32R, tag="rs",
                                 name=f"rs{blk}_{hh}")
                    nc.vector.tensor_copy(rs[:], pv[hh][64:65, :, :])
                    # broadcast rowsums across 64 partitions via K=1
                    # ones-matmul (PE), then reciprocal on DVE
                    rb = ps.tile([64, 2, 512], F32, tag="acc",
                                 name=f"rb{blk}_{hh}")
                    for ql in range(2):
                        nc.tensor.matmul(
                            rb[:, ql, :], ones_s[:], rs[:, ql, :],
                            start=True, stop=True,
                        )
                    rc = rp.tile([64, 2, 512], F32, tag="rc",
                                 name=f"rc{blk}_{hh}")
                    nc.vector.reciprocal_approx_fast(rc[:], rb[:])
                    if hh == 0:
                        nc.vector.tensor_mul(
                            ot[0:64, pp, :, :], pv[hh][0:64, :, :], rc[:]
                        )
                        nc.vector.tensor_scalar_add(
                            ot[0:64, pp, :, :], ot[0:64, pp, :, :],
                            bv_s[:, h:h + 1],
                        )
                    else:
                        tb = rp.tile([64, 2, 512], F32R, tag="tb",
                                     name=f"tb{blk}")
                        nc.vector.tensor_mul(tb[:], pv[hh][0:64, :, :], rc[:])
                        nc.vector.tensor_scalar_add(
                            tb[:], tb[:], bv_s[:, h:h + 1]
                        )
                        nc.sync.dma_start(ot[64:128, pp, :, :], tb[:])

            def oproj_group(qp, idx, og_act=False):
                ql, mt = idx // NDB, idx % NDB
                qc = qp * 2 + ql
                po = ps.tile([128, 512], F32, tag="acc",
                             name=f"po{qp}_{idx}")
                for fb in range(NFB):
                    nc.tensor.matmul(
                        po[:],
                        wo_s[:, fb, mt * 128:(mt + 1) * 128],
                        ot_t[qp][:, fb, ql, :],
                        start=(fb == 0),
                        stop=(fb == NFB - 1),
                    )
                og = osp.tile([128, 512], BF16, tag="og",
                              name=f"og{qp}_{idx}")
                if og_act:
                    nc.scalar.copy(og[:], po[:])
                else:
                    nc.vector.tensor_copy(og[:], po[:])
                nc.gpsimd.dma_start(out[mt][:, qc * 512:(qc + 1) * 512],
                                    og[:])

            # ---- interleaved emission schedule -------------------------
            # global iteration t = 0..63 drives scores/exp for block t//16;
            # PV+d, V-proj, Q-fb1, out-proj groups are drained as fillers.
            from collections import deque

            pv_pending = deque()     # (blk, kt) in order
            normalized = [False] * 4
            v_next = [0]             # next V-proj mt to emit
            op_pending = deque()     # (qp, idx) out-proj groups

            def vp_ready(kt, t):
                # V group for mt emitted at iter 7+mt; need 1 iter of margin
                return t >= 10 + kt

            def drain(t, budget):
                # returns PE-work units emitted
                n = 0
                while n < budget:
                    if pv_pending:
                        blk, kt = pv_pending[0]
                        prev_ok = blk == 0 or normalized[blk - 1] or \
                            pvp_t[blk] is not None
                        if vp_ready(kt, t) and prev_ok:
                            if pvp_t[blk] is None:
                                block_open(blk)
                            pv_pending.popleft()
                            pv_d(blk, kt)
                            n += 1
                            if kt == NKT - 1:
                                normalize(blk)
                                normalized[blk] = True
                                if blk == 3:
                                    pass
                            continue
                    if t >= 7 and v_next[0] < NMT:
                        v_group(v_next[0])
                        v_next[0] += 1
                        n += 1
                        continue
                    if op_pending and normalized[2 * op_pending[0][0] + 1]:
                        qp, idx = op_pending.popleft()
                        oproj_group(qp, idx, og_act=(idx % 2 == 1))
                        n += 1
                        continue
                    break
                return n

            for t in range(64):
                blk, kt = t // 16, t % 16
                qp, pp = blk // 2, blk % 2
                if t < len(proj_pending):
                    proj_group(*proj_pending[t])
                if kt == 0 and blk == 0:
                    block_open(0)
                scores_exp(blk, kt)
                pv_pending.append((blk, kt))
                if blk == 1 and kt == 15:
                    for i in range(2 * NDB):
                        op_pending.append((0, i))
                drain(t, 2 if t < 32 else (3 if t < 48 else 4))

            # ---- tail: finish remaining PV/normalize, out-proj qp=1 ----
            t = 64
            while pv_pending:
                drain(t, 4)
                t += 1
            while op_pending:
                qp, idx = op_pending.popleft()
                oproj_group(qp, idx)
            for i in range(2 * NDB):
                oproj_group(1, i, og_act=(i % 4 != 3))

    nc.compile()
    return nc


def _prep_core(b, g, query, key_, value, Wq, bq, Wk, bk, Wv, bv, Wo,
               xdt_name=XDT):
    """Host-side shard prep for core handling batch b, head group g."""
    import ml_dtypes

    fs = g * F
    f32 = np.float32
    xdt = ml_dtypes.bfloat16 if xdt_name == "bf16" else f32

    def xT(x):
        return np.ascontiguousarray(
            x[b].T.reshape(D // 128, 128, L), dtype=xdt
        )

    def xT8(x):
        a = x[b].T.reshape(D // 256, 2, 128, L).transpose(0, 2, 1, 3)
        return np.ascontiguousarray(a, dtype=ml_dtypes.float8_e4m3fn)

    def wT8(W):
        a = (32.0 * W[fs:fs + F, :].T).reshape(D // 128, 128, F)
        return np.ascontiguousarray(
            a.transpose(1, 0, 2), dtype=ml_dtypes.float8_e4m3fn)

    def wT(W):
        return np.ascontiguousarray(
            W[fs:fs + F, :].T.reshape(D // 128, 128, F).transpose(1, 0, 2),
            dtype=xdt,
        )

    wos = np.ascontiguousarray(
        Wo[:, fs:fs + F].T.reshape(F // 128, 128, D).transpose(1, 0, 2),
        dtype=f32,
    )
    bsc = 32.0 if FP8QK else 1.0
    return {
        "xq": xT8(query) if FP8QK else xT(query),
        "xk": xT8(key_) if FP8QK else xT(key_),
        "xv": xT(value),
        "wq": wT8(Wq) if FP8QK else wT(Wq),
        "wk": wT8(Wk) if FP8QK else wT(Wk),
        "wv": wT(Wv),
        "wo": wos,
        "bq": np.ascontiguousarray(
            (bsc * bq[fs:fs + F]).reshape(F // 128, 128).T, f32),
        "bk": np.ascontiguousarray(
            (bsc * bk[fs:fs + F]).reshape(F // 128, 128).T, f32),
        "bv": np.ascontiguousarray(bv[fs:fs + F].reshape(NH, 64).T, f32),
    }


def kernel(query, key_, value, Wq, bq, Wk, bk, Wv, bv, Wo, bo):
    from concourse.bass_utils import run_bass_kernel_spmd

    if "nc" not in _CACHE:
        _CACHE["nc"] = _build()
    nc = _CACHE["nc"]

    args = [np.asarray(a, np.float32) for a in
            (query, key_, value, Wq, bq, Wk, bk, Wv, bv, Wo)]
    in_maps = [_prep_core(c // 4, c % 4, *args) for c in range(NCORES)]
    res = run_bass_kernel_spmd(nc, in_maps, core_ids=list(range(NCORES)))
    global _LAST_EXEC_NS, _LAST_RES
    _LAST_EXEC_NS = getattr(res, "exec_time_ns", None)
    _LAST_RES = res

    final = np.zeros((B, L, D), np.float32)
    for c in range(NCORES):
        o = res.results[c]["out"]  # [8, 128, L] = partial out.T blocks
        final[c // 4] += o.reshape(D, L).T.astype(np.float32)
    final += np.asarray(bo, np.float32)
    return final
